# revision 16
# baseline (speedup 1.0000x reference)
"""Trainium2 Bass kernel for ConditionalEdgeDenoiser (GNN edge MLP denoiser).

Reference computation (per batch b, nodes i,j):
    h = concat([edge_t[b,i,j,:],            # 4   (EC)
                node_ctx[b,i,:],            # 80  (src = x_cond||code_cond)
                node_ctx[b,j,:],            # 80  (dst)
                time_emb[b,:]])             # 128 (TDIM)
    h1 = silu(h @ W1 + b1); h2 = silu(h1 @ W2 + b2); out = (h2 @ W3 + b3) * mask

Strategy (8 cores, data-parallel over (B x row-halves) = 8 shards of 128 rows):
  * Activations live as [hid on partitions, edge-columns on free dim]; a tile
    covers RPT=2 grid rows x 256 cols = 512 edges.
  * Layer 1 is ONE augmented matmul per hid-half: the stationary operand
    stacks [W1_edge (4) ; W1_dst (80) ; srcbias rows (2)] and the moving
    operand stacks [edge_T (4) ; node_ctx_T (80) ; row-indicators (2)].
    srcbias = node_ctx[i] @ W1_src + time_emb @ W1_time + b1 is precomputed
    on-device (transposed, per row i) and DMA'd into the stationary tile per
    tile iteration.  So PSUM holds the complete pre-activation and ScalarE
    does pure Silu over big contiguous tiles.
  * All matmul inputs are float32r (TF32-like, 1 row/cycle at N=512 vs 4x
    slower plain fp32); PSUM accumulates fp32.
"""

import os
import sys

sys.path.insert(0, "/opt/trn_rl_repo")
os.environ.setdefault("MYCRO_LOCAL_CACHE", "1")

import numpy as np

import concourse.bass as bass  # noqa: E402
import concourse.mybir as mybir  # noqa: E402
import concourse.tile as tile  # noqa: E402
from concourse import bacc  # noqa: E402
from concourse.bass_utils import run_bass_kernel_spmd  # noqa: E402

B, N, EC, FEAT, CODE, HID, TDIM = 4, 256, 4, 64, 16, 256, 128
NCTX = FEAT + CODE  # 80
NCORES = 8
RPT = 2                      # grid rows per tile
E = RPT * N                  # 512 edge columns per tile
ROWS = N // 2                # 128 grid rows per core
NT = ROWS // RPT             # 64 tiles per core
KAUG = EC + NCTX + RPT       # 86 = augmented contraction dim for layer 1
NPAIR = NT // 2              # 32 DMA pairs (2 compute tiles share one in/out DMA)

F32 = mybir.dt.float32
F32R = mybir.dt.float32r
AF = mybir.ActivationFunctionType
ALU = mybir.AluOpType

_CACHE = {}


def _build():
    nc = bacc.Bacc("TRN2", debug=False, num_devices=NCORES)

    # ---- DRAM I/O (per core) ----
    edge_d = nc.dram_tensor("edge", [NPAIR, EC, 2 * E], F32R, kind="ExternalInput")
    nctxT_d = nc.dram_tensor("nctxT", [NCTX, N], F32R, kind="ExternalInput")
    nctxTi_d = nc.dram_tensor("nctxTi", [NCTX, ROWS], F32R, kind="ExternalInput")
    tembT_d = nc.dram_tensor("tembT", [TDIM, 1], F32R, kind="ExternalInput")
    maskf_d = nc.dram_tensor("maskf", [N], F32, kind="ExternalInput")
    maskif_d = nc.dram_tensor("maskif", [ROWS], F32, kind="ExternalInput")
    w1e_d = nc.dram_tensor("w1e", [EC, HID], F32R, kind="ExternalInput")
    w1s_d = nc.dram_tensor("w1s", [NCTX, HID], F32R, kind="ExternalInput")
    w1d_d = nc.dram_tensor("w1d", [NCTX, HID], F32R, kind="ExternalInput")
    w1t_d = nc.dram_tensor("w1t", [TDIM, HID], F32R, kind="ExternalInput")
    b1_d = nc.dram_tensor("b1", [HID], F32R, kind="ExternalInput")
    b2c_d = nc.dram_tensor("b2c", [128, 2], F32, kind="ExternalInput")
    b3_d = nc.dram_tensor("b3", [EC], F32, kind="ExternalInput")
    w2_d = nc.dram_tensor("w2", [HID, HID], F32R, kind="ExternalInput")
    w3_d = nc.dram_tensor("w3", [HID, EC], F32R, kind="ExternalInput")
    ind_d = nc.dram_tensor("ind", [RPT, 2 * E], F32R, kind="ExternalInput")
    ones_d = nc.dram_tensor("ones", [1, 128], F32R, kind="ExternalInput")
    out_d = nc.dram_tensor("out", [NPAIR, EC, 2 * E], F32, kind="ExternalOutput")

    with tile.TileContext(nc) as tc:
        with tc.tile_pool(name="const", bufs=1) as cp, \
             tc.tile_pool(name="h", bufs=3) as hp, \
             tc.tile_pool(name="o", bufs=3) as op, \
             tc.tile_pool(name="ps", bufs=2, space="PSUM") as pp:

            # ---------- constant loads ----------
            w2k0 = cp.tile([128, HID], F32R, tag="w2k0")
            nc.sync.dma_start(out=w2k0, in_=w2_d[0:128])
            w2k1 = cp.tile([128, HID], F32R, tag="w2k1")
            nc.sync.dma_start(out=w2k1, in_=w2_d[128:256])
            w30 = cp.tile([128, EC], F32R, tag="w30")
            nc.sync.dma_start(out=w30, in_=w3_d[0:128])
            w31 = cp.tile([128, EC], F32R, tag="w31")
            nc.sync.dma_start(out=w31, in_=w3_d[128:256])
            b2c = cp.tile([128, 2], F32, tag="b2c")
            nc.sync.dma_start(out=b2c, in_=b2c_d[:])
            b3c = cp.tile([EC, 1], F32, tag="b3c")
            nc.sync.dma_start(out=b3c, in_=b3_d[:].rearrange("(p o) -> p o", o=1))

            w1t_s = cp.tile([TDIM, HID], F32R, tag="w1t")
            nc.sync.dma_start(out=w1t_s, in_=w1t_d[:])
            w1s_s = cp.tile([NCTX, HID], F32R, tag="w1s")
            nc.sync.dma_start(out=w1s_s, in_=w1s_d[:])
            b1r = cp.tile([1, HID], F32R, tag="b1r")
            nc.sync.dma_start(out=b1r, in_=b1_d[:].rearrange("(o f) -> o f", o=1))
            tembT_s = cp.tile([TDIM, 1], F32R, tag="tembT")
            nc.sync.dma_start(out=tembT_s, in_=tembT_d[:])
            nctxTi_s = cp.tile([NCTX, ROWS], F32R, tag="nctxTi")
            nc.sync.dma_start(out=nctxTi_s, in_=nctxTi_d[:])
            onesc = cp.tile([1, 128], F32R, tag="onesc")
            nc.sync.dma_start(out=onesc, in_=ones_d[:])

            # mask broadcast tiles: mask4[c, r*N+j] = maskf[j]; mif4[c, i] = maskif[i]
            mask4 = cp.tile([EC, E], F32, tag="mask4")
            for r in range(RPT):
                src = bass.AP(tensor=maskf_d[:].tensor, offset=maskf_d[:].offset,
                              ap=[[0, EC], [1, N]])
                nc.sync.dma_start(out=mask4[:, r * N:(r + 1) * N], in_=src)
            mif4 = cp.tile([EC, ROWS], F32, tag="mif4")
            nc.sync.dma_start(
                out=mif4,
                in_=bass.AP(tensor=maskif_d[:].tensor, offset=maskif_d[:].offset,
                            ap=[[0, EC], [1, ROWS]]))

            # ---------- srcbiasT precompute ----------
            # tbrow[0, h] = temb @ W1t + b1
            ps_tb = pp.tile([1, HID], F32, tag="p2")
            nc.tensor.matmul(ps_tb, lhsT=tembT_s, rhs=w1t_s, start=True, stop=False)
            nc.tensor.matmul(ps_tb, lhsT=onesc[:, 0:1], rhs=b1r, start=False, stop=True)
            tbrow = cp.tile([1, HID], F32R, tag="tbrow")
            nc.scalar.activation(tbrow, ps_tb, AF.Copy)
            # srcbT[i, h] = node_ctx[i] @ W1s + tbrow
            ps_sb = pp.tile([ROWS, HID], F32, tag="p2")
            nc.tensor.matmul(ps_sb, lhsT=nctxTi_s, rhs=w1s_s, start=True, stop=False)
            nc.tensor.matmul(ps_sb, lhsT=onesc, rhs=tbrow, start=False, stop=True)
            srcbT = cp.tile([ROWS, HID], F32R, tag="srcbT")
            nc.scalar.activation(srcbT, ps_sb, AF.Copy)

            # ---------- augmented layer-1 operands (ping-pong pairs) ----------
            # lh[q]: [KAUG, 256] stationary tile, halves at cols 0:128 / 128:256.
            # rhs_t[q]: [KAUG, 2E] moving tile holding TWO compute tiles.
            lh = [None, None]
            rhs_t = [None, None]
            for q in range(2):
                lt = cp.tile([KAUG, HID], F32R, tag=f"lh{q}")
                nc.sync.dma_start(out=lt[0:EC, :], in_=w1e_d[:])
                nc.sync.dma_start(out=lt[EC:EC + NCTX, :], in_=w1d_d[:])
                lh[q] = lt
                rt = cp.tile([KAUG, 2 * E], F32R, tag=f"rhs{q}")
                for r in range(2 * RPT):
                    nc.sync.dma_start(out=rt[EC:EC + NCTX, r * N:(r + 1) * N],
                                      in_=nctxT_d[:])
                nc.sync.dma_start(out=rt[EC + NCTX:KAUG, :], in_=ind_d[:])
                rhs_t[q] = rt

            # ---------- main loop: 32 DMA pairs x 2 compute tiles ----------
            for s in range(NPAIR):
                rhs = rhs_t[s % 2]
                nc.sync.dma_start(out=rhs[0:EC, :], in_=edge_d[s])
                ot = op.tile([EC, 2 * E], F32, tag="ot")
                for u in range(2):
                    t = 2 * s + u
                    lht = lh[t % 2]
                    # per-tile srcbias rows -> stationary tile (SWDGE, Pool engine)
                    nc.gpsimd.dma_start(out=lht[EC + NCTX:KAUG, :],
                                        in_=srcbT[RPT * t:RPT * (t + 1), :])
                    rhs_u = rhs[:, u * E:(u + 1) * E]

                    p1 = pp.tile([128, 2 * E], F32, tag="p13")
                    nc.tensor.matmul(p1[:, 0:E], lhsT=lht[:, 0:128], rhs=rhs_u,
                                     start=True, stop=True)
                    nc.tensor.matmul(p1[:, E:2 * E], lhsT=lht[:, 128:256], rhs=rhs_u,
                                     start=True, stop=True)

                    h1 = hp.tile([128, 2 * E], F32R, tag="h1")
                    nc.scalar.activation(h1, p1, AF.Silu)

                    p2 = pp.tile([128, 2 * E], F32, tag="p2")
                    nc.tensor.matmul(p2[:, 0:E], lhsT=w2k0[:, 0:128], rhs=h1[:, 0:E],
                                     start=True, stop=False)
                    nc.tensor.matmul(p2[:, 0:E], lhsT=w2k1[:, 0:128], rhs=h1[:, E:2 * E],
                                     start=False, stop=True)
                    nc.tensor.matmul(p2[:, E:2 * E], lhsT=w2k0[:, 128:256],
                                     rhs=h1[:, 0:E], start=True, stop=False)
                    nc.tensor.matmul(p2[:, E:2 * E], lhsT=w2k1[:, 128:256],
                                     rhs=h1[:, E:2 * E], start=False, stop=True)

                    h2 = hp.tile([128, 2 * E], F32R, tag="h2")
                    nc.scalar.activation(h2[:, 0:E], p2[:, 0:E], AF.Silu,
                                         bias=b2c[:, 0:1])
                    nc.scalar.activation(h2[:, E:2 * E], p2[:, E:2 * E], AF.Silu,
                                         bias=b2c[:, 1:2])

                    p3 = pp.tile([EC, E], F32, tag="p13")
                    nc.tensor.matmul(p3, lhsT=w30, rhs=h2[:, 0:E], start=True, stop=False)
                    nc.tensor.matmul(p3, lhsT=w31, rhs=h2[:, E:2 * E],
                                     start=False, stop=True)

                    for r in range(RPT):
                        # (p3 + b3) * mask_i  for this grid row
                        nc.vector.tensor_scalar(
                            out=ot[:, u * E + r * N:u * E + (r + 1) * N],
                            in0=p3[:, r * N:(r + 1) * N],
                            scalar1=b3c, scalar2=mif4[:, RPT * t + r:RPT * t + r + 1],
                            op0=ALU.add, op1=ALU.mult)
                    nc.vector.tensor_mul(out=ot[:, u * E:(u + 1) * E],
                                         in0=ot[:, u * E:(u + 1) * E], in1=mask4)
                nc.sync.dma_start(out=out_d[s], in_=ot)

    nc.compile()
    return nc


def _get_nc():
    if "nc" not in _CACHE:
        _CACHE["nc"] = _build()
    return _CACHE["nc"]


def _time_embedding(t):
    half = TDIM // 2
    freqs = np.exp(-np.arange(half, dtype=np.float32)
                   * (np.float32(np.log(10000.0)) / np.float32(half - 1)))
    args = np.asarray(t).astype(np.float32)[:, None] * freqs[None, :]
    return np.concatenate([np.sin(args), np.cos(args)], axis=1).astype(np.float32)


def _prepare_in_maps(edge_t, x_cond, code_cond, t, node_mask, W1, b1, W2, b2, W3, b3):
    edge_t = np.ascontiguousarray(np.asarray(edge_t, dtype=np.float32))
    node_ctx = np.concatenate(
        [np.asarray(x_cond, np.float32), np.asarray(code_cond, np.float32)], axis=-1)
    temb = _time_embedding(t)                       # [B, TDIM]
    maskf = np.asarray(node_mask).astype(np.float32)  # [B, N]
    W1 = np.asarray(W1, np.float32)
    w1e = np.ascontiguousarray(W1[0:EC])
    w1s = np.ascontiguousarray(W1[EC:EC + NCTX])
    w1d = np.ascontiguousarray(W1[EC + NCTX:EC + 2 * NCTX])
    w1t = np.ascontiguousarray(W1[EC + 2 * NCTX:])
    b1 = np.asarray(b1, np.float32)
    b2c = np.ascontiguousarray(np.asarray(b2, np.float32).reshape(2, 128).T)
    b3 = np.asarray(b3, np.float32)
    W2 = np.ascontiguousarray(np.asarray(W2, np.float32))
    W3 = np.ascontiguousarray(np.asarray(W3, np.float32))

    in_maps = []
    for c in range(NCORES):
        b, ih = c // 2, c % 2
        i0 = ih * ROWS
        es = edge_t[b, i0:i0 + ROWS]               # [ROWS, N, EC]
        # pair layout: [NPAIR, EC, (u r j)] with u=tile-in-pair, r=row-in-tile
        er = np.ascontiguousarray(
            es.reshape(NPAIR, 2, RPT, N, EC).transpose(0, 4, 1, 2, 3)
            .reshape(NPAIR, EC, 2 * E))
        in_maps.append({
            "edge": er,
            "nctxT": np.ascontiguousarray(node_ctx[b].T),
            "nctxTi": np.ascontiguousarray(node_ctx[b, i0:i0 + ROWS].T),
            "tembT": np.ascontiguousarray(temb[b][:, None]),
            "maskf": np.ascontiguousarray(maskf[b]),
            "maskif": np.ascontiguousarray(maskf[b, i0:i0 + ROWS]),
            "w1e": w1e, "w1s": w1s, "w1d": w1d, "w1t": w1t,
            "b1": b1, "b2c": b2c, "b3": b3, "w2": W2, "w3": W3,
            "ind": _indicator(),
            "ones": np.ones((1, 128), dtype=np.float32),
        })
    return in_maps


def _indicator():
    ind = np.zeros((RPT, E), dtype=np.float32)
    for r in range(RPT):
        ind[r, r * N:(r + 1) * N] = 1.0
    return np.ascontiguousarray(np.tile(ind, (1, 2)))  # [RPT, 2E]


def _assemble(results):
    out = np.empty((B, N, N, EC), dtype=np.float32)
    for c in range(NCORES):
        b, ih = c // 2, c % 2
        i0 = ih * ROWS
        o = results[c]["out"]                      # [NPAIR, EC, 2E]
        out[b, i0:i0 + ROWS] = (
            o.reshape(NPAIR, EC, 2, RPT, N).transpose(0, 2, 3, 4, 1)
            .reshape(ROWS, N, EC))
    return out


def _run(in_maps, trace=False, **kwargs):
    nc = _get_nc()
    return run_bass_kernel_spmd(nc, in_maps, list(range(NCORES)), trace=trace, **kwargs)


def kernel(**inputs):
    in_maps = _prepare_in_maps(**inputs)
    res = _run(in_maps)
    return _assemble(res.results)


def kernel_profiled(**inputs):
    """Like kernel() but also returns (output, exec_time_ns)."""
    in_maps = _prepare_in_maps(**inputs)
    res = _run(in_maps, trace=True)
    return _assemble(res.results), res.exec_time_ns


# revision 18
# speedup vs baseline: 1.5213x; 1.5213x over previous
"""Trainium2 Bass kernel for ConditionalEdgeDenoiser (GNN edge MLP denoiser).

Reference computation (per batch b, nodes i,j):
    h = concat([edge_t[b,i,j,:],            # 4   (EC)
                node_ctx[b,i,:],            # 80  (src = x_cond||code_cond)
                node_ctx[b,j,:],            # 80  (dst)
                time_emb[b,:]])             # 128 (TDIM)
    h1 = silu(h @ W1 + b1); h2 = silu(h1 @ W2 + b2); out = (h2 @ W3 + b3) * mask

Strategy (8 cores, data-parallel over (B x row-halves) = 8 shards of 128 rows):
  * Activations live as [hid on partitions, edge-columns on free dim]; a tile
    covers RPT=2 grid rows x 256 cols = 512 edges.
  * Layer 1 is ONE augmented matmul per hid-half: the stationary operand
    stacks [W1_edge (4) ; W1_dst (80) ; srcbias rows (2)] and the moving
    operand stacks [edge_T (4) ; node_ctx_T (80) ; row-indicators (2)].
    srcbias = node_ctx[i] @ W1_src + time_emb @ W1_time + b1 is precomputed
    on-device (transposed, per row i) and DMA'd into the stationary tile per
    tile iteration.  So PSUM holds the complete pre-activation and ScalarE
    does pure Silu over big contiguous tiles.
  * All matmul inputs are float32r (TF32-like, 1 row/cycle at N=512 vs 4x
    slower plain fp32); PSUM accumulates fp32.
"""

import os
import sys

sys.path.insert(0, "/opt/trn_rl_repo")
os.environ.setdefault("MYCRO_LOCAL_CACHE", "1")

import numpy as np

import concourse.bass as bass  # noqa: E402
import concourse.mybir as mybir  # noqa: E402
import concourse.tile as tile  # noqa: E402
from concourse import bacc  # noqa: E402
from concourse.bass_utils import run_bass_kernel_spmd  # noqa: E402

B, N, EC, FEAT, CODE, HID, TDIM = 4, 256, 4, 64, 16, 256, 128
NCTX = FEAT + CODE  # 80
NCORES = 8
RPT = 2                      # grid rows per tile
E = RPT * N                  # 512 edge columns per tile
ROWS = N // 2                # 128 grid rows per core
NT = ROWS // RPT             # 64 tiles per core
KAUG = EC + NCTX + RPT       # 86 = augmented contraction dim for layer 1
NPAIR = NT // 2              # 32 DMA pairs (2 compute tiles share one in/out DMA)

F32 = mybir.dt.float32
F32R = mybir.dt.float32r
AF = mybir.ActivationFunctionType
ALU = mybir.AluOpType

_CACHE = {}


def _build():
    nc = bacc.Bacc("TRN2", debug=False, num_devices=NCORES)

    # ---- DRAM I/O (per core) ----
    edge_d = nc.dram_tensor("edge", [NPAIR, EC, 2 * E], F32R, kind="ExternalInput")
    nctxT_d = nc.dram_tensor("nctxT", [NCTX, N], F32R, kind="ExternalInput")
    nctxTi_d = nc.dram_tensor("nctxTi", [NCTX, ROWS], F32R, kind="ExternalInput")
    tembT_d = nc.dram_tensor("tembT", [TDIM, 1], F32R, kind="ExternalInput")
    maskf_d = nc.dram_tensor("maskf", [N], F32, kind="ExternalInput")
    maskif_d = nc.dram_tensor("maskif", [ROWS], F32, kind="ExternalInput")
    w1e_d = nc.dram_tensor("w1e", [EC, HID], F32R, kind="ExternalInput")
    w1s_d = nc.dram_tensor("w1s", [NCTX, HID], F32R, kind="ExternalInput")
    w1d_d = nc.dram_tensor("w1d", [NCTX, HID], F32R, kind="ExternalInput")
    w1t_d = nc.dram_tensor("w1t", [TDIM, HID], F32R, kind="ExternalInput")
    b1_d = nc.dram_tensor("b1", [HID], F32R, kind="ExternalInput")
    b2c_d = nc.dram_tensor("b2c", [128, 2], F32, kind="ExternalInput")
    b3_d = nc.dram_tensor("b3", [EC], F32, kind="ExternalInput")
    w2_d = nc.dram_tensor("w2", [HID, HID], F32R, kind="ExternalInput")
    w3_d = nc.dram_tensor("w3", [HID, EC], F32R, kind="ExternalInput")
    ind_d = nc.dram_tensor("ind", [RPT, 2 * E], F32R, kind="ExternalInput")
    ones_d = nc.dram_tensor("ones", [1, 128], F32R, kind="ExternalInput")
    out_d = nc.dram_tensor("out", [NPAIR, EC, 2 * E], F32, kind="ExternalOutput")

    with tile.TileContext(nc) as tc:
        with tc.tile_pool(name="const", bufs=1) as cp, \
             tc.tile_pool(name="h", bufs=3) as hp, \
             tc.tile_pool(name="o", bufs=3) as op, \
             tc.tile_pool(name="ps", bufs=2, space="PSUM") as pp:

            # ---------- constant loads ----------
            w2k0 = cp.tile([128, HID], F32R, tag="w2k0")
            nc.sync.dma_start(out=w2k0, in_=w2_d[0:128])
            w2k1 = cp.tile([128, HID], F32R, tag="w2k1")
            nc.sync.dma_start(out=w2k1, in_=w2_d[128:256])
            w30 = cp.tile([128, EC], F32R, tag="w30")
            nc.sync.dma_start(out=w30, in_=w3_d[0:128])
            w31 = cp.tile([128, EC], F32R, tag="w31")
            nc.sync.dma_start(out=w31, in_=w3_d[128:256])
            b2c = cp.tile([128, 2], F32, tag="b2c")
            nc.sync.dma_start(out=b2c, in_=b2c_d[:])
            b3c = cp.tile([EC, 1], F32, tag="b3c")
            nc.sync.dma_start(out=b3c, in_=b3_d[:].rearrange("(p o) -> p o", o=1))

            w1t_s = cp.tile([TDIM, HID], F32R, tag="w1t")
            nc.sync.dma_start(out=w1t_s, in_=w1t_d[:])
            w1s_s = cp.tile([NCTX, HID], F32R, tag="w1s")
            nc.sync.dma_start(out=w1s_s, in_=w1s_d[:])
            b1r = cp.tile([1, HID], F32R, tag="b1r")
            nc.sync.dma_start(out=b1r, in_=b1_d[:].rearrange("(o f) -> o f", o=1))
            tembT_s = cp.tile([TDIM, 1], F32R, tag="tembT")
            nc.sync.dma_start(out=tembT_s, in_=tembT_d[:])
            nctxTi_s = cp.tile([NCTX, ROWS], F32R, tag="nctxTi")
            nc.sync.dma_start(out=nctxTi_s, in_=nctxTi_d[:])
            onesc = cp.tile([1, 128], F32R, tag="onesc")
            nc.sync.dma_start(out=onesc, in_=ones_d[:])

            # mask broadcast tiles: mask4[c, r*N+j] = maskf[j]; mif4[c, i] = maskif[i]
            mask4 = cp.tile([EC, E], F32, tag="mask4")
            for r in range(RPT):
                src = bass.AP(tensor=maskf_d[:].tensor, offset=maskf_d[:].offset,
                              ap=[[0, EC], [1, N]])
                nc.sync.dma_start(out=mask4[:, r * N:(r + 1) * N], in_=src)
            mif4 = cp.tile([EC, ROWS], F32, tag="mif4")
            nc.sync.dma_start(
                out=mif4,
                in_=bass.AP(tensor=maskif_d[:].tensor, offset=maskif_d[:].offset,
                            ap=[[0, EC], [1, ROWS]]))

            # ---------- srcbiasT precompute ----------
            # tbrow[0, h] = temb @ W1t + b1
            ps_tb = pp.tile([1, HID], F32, tag="p2")
            nc.tensor.matmul(ps_tb, lhsT=tembT_s, rhs=w1t_s, start=True, stop=False)
            nc.tensor.matmul(ps_tb, lhsT=onesc[:, 0:1], rhs=b1r, start=False, stop=True)
            tbrow = cp.tile([1, HID], F32R, tag="tbrow")
            nc.scalar.activation(tbrow, ps_tb, AF.Copy)
            # srcbT[i, h] = node_ctx[i] @ W1s + tbrow
            ps_sb = pp.tile([ROWS, HID], F32, tag="p2")
            nc.tensor.matmul(ps_sb, lhsT=nctxTi_s, rhs=w1s_s, start=True, stop=False)
            nc.tensor.matmul(ps_sb, lhsT=onesc, rhs=tbrow, start=False, stop=True)
            srcbT = cp.tile([ROWS, HID], F32R, tag="srcbT")
            nc.scalar.activation(srcbT, ps_sb, AF.Copy)

            # ---------- augmented layer-1 operands (ping-pong pairs) ----------
            # lh[q]: [KAUG, 256] stationary tile, halves at cols 0:128 / 128:256.
            # rhs_t[q]: [KAUG, 2E] moving tile holding TWO compute tiles.
            lh = [None, None]
            rhs_t = [None, None]
            for q in range(2):
                lt = cp.tile([KAUG, HID], F32R, tag=f"lh{q}")
                nc.sync.dma_start(out=lt[0:EC, :], in_=w1e_d[:])
                nc.sync.dma_start(out=lt[EC:EC + NCTX, :], in_=w1d_d[:])
                lh[q] = lt
                rt = cp.tile([KAUG, 2 * E], F32R, tag=f"rhs{q}")
                for r in range(2 * RPT):
                    nc.sync.dma_start(out=rt[EC:EC + NCTX, r * N:(r + 1) * N],
                                      in_=nctxT_d[:])
                nc.sync.dma_start(out=rt[EC + NCTX:KAUG, :], in_=ind_d[:])
                rhs_t[q] = rt

            # ---------- main loop: 3-stage software pipeline ----------
            # iteration k emits: L1+silu1 for tile k, L2+silu2 for tile k-1,
            # L3+mask+out for tile k-2 — so each engine's static instruction
            # order interleaves adjacent tiles and never stalls on the
            # silu1 -> L2 -> silu2 chain of a single tile.
            h1s, h2s, p3s, ots = {}, {}, {}, {}
            for k in range(NT + 2):
                if k < NT:
                    s, u = divmod(k, 2)
                    rhs = rhs_t[s % 2]
                    if u == 0:
                        nc.sync.dma_start(out=rhs[0:EC, :], in_=edge_d[s])
                    lht = lh[k % 2]
                    # per-tile srcbias rows -> stationary tile (SWDGE, Pool)
                    nc.gpsimd.dma_start(out=lht[EC + NCTX:KAUG, :],
                                        in_=srcbT[RPT * k:RPT * (k + 1), :])
                    rhs_u = rhs[:, u * E:(u + 1) * E]
                    p1 = pp.tile([128, 2 * E], F32, tag="p13")
                    nc.tensor.matmul(p1[:, 0:E], lhsT=lht[:, 0:128], rhs=rhs_u,
                                     start=True, stop=True)
                    nc.tensor.matmul(p1[:, E:2 * E], lhsT=lht[:, 128:256], rhs=rhs_u,
                                     start=True, stop=True)
                    h1 = hp.tile([128, 2 * E], F32R, tag="h1")
                    nc.scalar.activation(h1, p1, AF.Silu)
                    h1s[k] = h1

                if 1 <= k <= NT:
                    j = k - 1
                    h1 = h1s.pop(j)
                    p2 = pp.tile([128, 2 * E], F32, tag="p2")
                    nc.tensor.matmul(p2[:, 0:E], lhsT=w2k0[:, 0:128], rhs=h1[:, 0:E],
                                     start=True, stop=False)
                    nc.tensor.matmul(p2[:, 0:E], lhsT=w2k1[:, 0:128], rhs=h1[:, E:2 * E],
                                     start=False, stop=True)
                    nc.tensor.matmul(p2[:, E:2 * E], lhsT=w2k0[:, 128:256],
                                     rhs=h1[:, 0:E], start=True, stop=False)
                    nc.tensor.matmul(p2[:, E:2 * E], lhsT=w2k1[:, 128:256],
                                     rhs=h1[:, E:2 * E], start=False, stop=True)
                    h2 = hp.tile([128, 2 * E], F32R, tag="h2")
                    nc.scalar.activation(h2[:, 0:E], p2[:, 0:E], AF.Silu,
                                         bias=b2c[:, 0:1])
                    nc.scalar.activation(h2[:, E:2 * E], p2[:, E:2 * E], AF.Silu,
                                         bias=b2c[:, 1:2])
                    h2s[j] = h2

                if k >= 2:
                    i = k - 2
                    si, ui = divmod(i, 2)
                    h2 = h2s.pop(i)
                    p3 = pp.tile([EC, E], F32, tag="p13")
                    nc.tensor.matmul(p3, lhsT=w30, rhs=h2[:, 0:E], start=True, stop=False)
                    nc.tensor.matmul(p3, lhsT=w31, rhs=h2[:, E:2 * E],
                                     start=False, stop=True)
                    if ui == 0:
                        ots[si] = op.tile([EC, 2 * E], F32, name=f"ot{si}", tag="ot")
                    ot = ots[si]
                    for r in range(RPT):
                        # (p3 + b3) * mask_i  for this grid row
                        nc.vector.tensor_scalar(
                            out=ot[:, ui * E + r * N:ui * E + (r + 1) * N],
                            in0=p3[:, r * N:(r + 1) * N],
                            scalar1=b3c, scalar2=mif4[:, RPT * i + r:RPT * i + r + 1],
                            op0=ALU.add, op1=ALU.mult)
                    nc.vector.tensor_mul(out=ot[:, ui * E:(ui + 1) * E],
                                         in0=ot[:, ui * E:(ui + 1) * E], in1=mask4)
                    if ui == 1:
                        nc.sync.dma_start(out=out_d[si], in_=ots.pop(si))

    nc.compile()
    return nc


def _get_nc():
    if "nc" not in _CACHE:
        _CACHE["nc"] = _build()
    return _CACHE["nc"]


def _time_embedding(t):
    half = TDIM // 2
    freqs = np.exp(-np.arange(half, dtype=np.float32)
                   * (np.float32(np.log(10000.0)) / np.float32(half - 1)))
    args = np.asarray(t).astype(np.float32)[:, None] * freqs[None, :]
    return np.concatenate([np.sin(args), np.cos(args)], axis=1).astype(np.float32)


def _prepare_in_maps(edge_t, x_cond, code_cond, t, node_mask, W1, b1, W2, b2, W3, b3):
    edge_t = np.ascontiguousarray(np.asarray(edge_t, dtype=np.float32))
    node_ctx = np.concatenate(
        [np.asarray(x_cond, np.float32), np.asarray(code_cond, np.float32)], axis=-1)
    temb = _time_embedding(t)                       # [B, TDIM]
    maskf = np.asarray(node_mask).astype(np.float32)  # [B, N]
    W1 = np.asarray(W1, np.float32)
    w1e = np.ascontiguousarray(W1[0:EC])
    w1s = np.ascontiguousarray(W1[EC:EC + NCTX])
    w1d = np.ascontiguousarray(W1[EC + NCTX:EC + 2 * NCTX])
    w1t = np.ascontiguousarray(W1[EC + 2 * NCTX:])
    b1 = np.asarray(b1, np.float32)
    b2c = np.ascontiguousarray(np.asarray(b2, np.float32).reshape(2, 128).T)
    b3 = np.asarray(b3, np.float32)
    W2 = np.ascontiguousarray(np.asarray(W2, np.float32))
    W3 = np.ascontiguousarray(np.asarray(W3, np.float32))

    in_maps = []
    for c in range(NCORES):
        b, ih = c // 2, c % 2
        i0 = ih * ROWS
        es = edge_t[b, i0:i0 + ROWS]               # [ROWS, N, EC]
        # pair layout: [NPAIR, EC, (u r j)] with u=tile-in-pair, r=row-in-tile
        er = np.ascontiguousarray(
            es.reshape(NPAIR, 2, RPT, N, EC).transpose(0, 4, 1, 2, 3)
            .reshape(NPAIR, EC, 2 * E))
        in_maps.append({
            "edge": er,
            "nctxT": np.ascontiguousarray(node_ctx[b].T),
            "nctxTi": np.ascontiguousarray(node_ctx[b, i0:i0 + ROWS].T),
            "tembT": np.ascontiguousarray(temb[b][:, None]),
            "maskf": np.ascontiguousarray(maskf[b]),
            "maskif": np.ascontiguousarray(maskf[b, i0:i0 + ROWS]),
            "w1e": w1e, "w1s": w1s, "w1d": w1d, "w1t": w1t,
            "b1": b1, "b2c": b2c, "b3": b3, "w2": W2, "w3": W3,
            "ind": _indicator(),
            "ones": np.ones((1, 128), dtype=np.float32),
        })
    return in_maps


def _indicator():
    ind = np.zeros((RPT, E), dtype=np.float32)
    for r in range(RPT):
        ind[r, r * N:(r + 1) * N] = 1.0
    return np.ascontiguousarray(np.tile(ind, (1, 2)))  # [RPT, 2E]


def _assemble(results):
    out = np.empty((B, N, N, EC), dtype=np.float32)
    for c in range(NCORES):
        b, ih = c // 2, c % 2
        i0 = ih * ROWS
        o = results[c]["out"]                      # [NPAIR, EC, 2E]
        out[b, i0:i0 + ROWS] = (
            o.reshape(NPAIR, EC, 2, RPT, N).transpose(0, 2, 3, 4, 1)
            .reshape(ROWS, N, EC))
    return out


def _run(in_maps, trace=False, **kwargs):
    nc = _get_nc()
    return run_bass_kernel_spmd(nc, in_maps, list(range(NCORES)), trace=trace, **kwargs)


def kernel(**inputs):
    in_maps = _prepare_in_maps(**inputs)
    res = _run(in_maps)
    return _assemble(res.results)


def kernel_profiled(**inputs):
    """Like kernel() but also returns (output, exec_time_ns)."""
    in_maps = _prepare_in_maps(**inputs)
    res = _run(in_maps, trace=True)
    return _assemble(res.results), res.exec_time_ns
